# revision 18
# baseline (speedup 1.0000x reference)
"""Trainium2 Bass kernel for nn_AttentionCell (sparse local attention, W=16).

Contract: kernel(**inputs) takes the FULL inputs
    inputs: [8, 1024, 512] f32, M/C/V: [512, 512] f32
and returns the FULL output [8, 1024, 1024] f32
    out = concat([inputs, local_attention(inputs)], axis=-1)

Sharding: data-parallel over batch — one batch element per NeuronCore (8 cores).
M/C are fused on the host into G = M @ C.T so that
    logits = (x @ M) @ (x @ C).T = (x @ G) @ x.T
which removes the K projection entirely on device (keys are x itself).

Per-core device algorithm (x: [1024, 512]):
  1. xT = x.T via PE transposes, stored zero-padded by LEFT-1=15 columns on the
     left so every 128-query chunk's 143-wide key span is a contiguous slice.
  2. Q'T = G.T @ xT and Vn = x @ Vw as float32r matmuls (1 cyc/row on PE).
  3. Per 128-query chunk: banded logits L[i, j] (j over the 143-key span) as
     4 accumulating matmuls; softmax over the in-band 16 entries via an
     additive -1e9 band mask (out-of-sequence keys are zero columns of xT so
     their logits are exactly 0, matching the reference's zero-padding).
  4. S @ V via two matmuls (15-row tail from the previous V chunk + the
     aligned 128-row chunk) after transposing the scores on PE; the softmax
     normalization is folded into the PSUM->SBUF copy as a per-row scale.
"""

import os
import sys

import numpy as np

for _p in ("/opt/trn_rl_repo", "/opt/pypackages"):
    if os.path.isdir(_p) and _p not in sys.path:
        sys.path.append(_p)

import concourse.bacc as bacc
import concourse.tile as tile
from concourse import mybir
from concourse.bass_utils import run_bass_kernel_spmd
from concourse.masks import make_identity

f32 = mybir.dt.float32
f32r = mybir.dt.float32r

B = 8
T = 1024
D = 512
LEFT = 16
PAD = LEFT - 1  # 15
# fp32r matmuls require even innermost free counts (2 fp32 per cycle), so the
# per-chunk key span is padded 143 -> 144; the extra column is band-masked.
SPAN = 128 + PAD + 1  # 144
XTW = PAD + T + 1  # padded xT width: 15 zero cols left, 1 zero col right
NCH = T // 128  # query chunks per core
NDC = D // 128  # feature chunks
MASKVAL = -1.0e9

_cache: dict = {}


def _ts(i, n=128):
    return slice(i * n, (i + 1) * n)


def _emit(tc, nc, xd, Gd, Vd, Bd, Zd, outd):
    AF = mybir.ActivationFunctionType
    OP = mybir.AluOpType
    from contextlib import ExitStack

    stack = ExitStack()
    constp = stack.enter_context(tc.tile_pool(name="const", bufs=1))
    xinp = stack.enter_context(tc.tile_pool(name="xin", bufs=3))
    bigp = stack.enter_context(tc.tile_pool(name="big", bufs=1))
    smp = stack.enter_context(tc.tile_pool(name="sm", bufs=2))
    pTp = stack.enter_context(tc.tile_pool(name="pT", bufs=2, space="PSUM"))
    pQVp = stack.enter_context(tc.tile_pool(name="pQV", bufs=2, space="PSUM"))
    pLp = stack.enter_context(tc.tile_pool(name="pL", bufs=2, space="PSUM"))
    pAp = stack.enter_context(tc.tile_pool(name="pA", bufs=2, space="PSUM"))

    # --- constants / weights ---
    identity = constp.tile([128, 128], f32)
    make_identity(nc, identity[:])
    band = constp.tile([128, SPAN], f32)
    nc.sync.dma_start(band[:], Bd[:])
    Gw = constp.tile([128, NDC, D], f32r)
    nc.sync.dma_start(Gw[:], Gd[:].rearrange("(c p) n -> p c n", p=128).bitcast(f32r))
    Vws = constp.tile([128, NDC, D], f32r)
    nc.sync.dma_start(Vws[:], Vd[:].rearrange("(c p) n -> p c n", p=128).bitcast(f32r))

    # --- persistent activations ---
    # x.T, zero-padded: cols 0..14 (left halo) and col XTW-1 (right span pad).
    # memset cannot write float32r on trn2, so zero-fill comes from a small
    # DMA'd zeros input instead.
    xTp = bigp.tile([128, NDC, XTW], f32r)
    nc.sync.dma_start(
        xTp[:, :, 0:PAD],
        Zd[:, 0 : NDC * PAD].rearrange("p (c t) -> p c t", c=NDC).bitcast(f32r),
    )
    nc.sync.dma_start(
        xTp[:, :, XTW - 1 : XTW],
        Zd[:, 0:NDC].rearrange("p (c t) -> p c t", c=NDC).bitcast(f32r),
    )
    QT = bigp.tile([128, NDC, T], f32r)  # (x @ G).T
    Vn = bigp.tile([128, NCH, D], f32r)  # x @ Vw, natural layout
    Vtail = bigp.tile([PAD, NCH, D], f32r)  # V rows t0-15..t0-1 per chunk
    nc.sync.dma_start(Vtail[:, 0, :], Zd[0:PAD, :].bitcast(f32r))

    # --- load + transpose x (and passthrough copy) ---
    def load_transpose(i):
        xn = xinp.tile([128, D], f32, name=f"xn{i}", tag="xn")
        nc.sync.dma_start(xn[:], xd[_ts(i), :])
        pst = pTp.tile([128, D], f32, name=f"pt{i}", tag="pt")
        for dc in range(NDC):
            nc.tensor.transpose(pst[:, _ts(dc)], xn[:, _ts(dc)], identity[:])
        nc.vector.tensor_copy(
            xTp[:, :, PAD + 128 * i : PAD + 128 * (i + 1)],
            pst[:].rearrange("p (c t) -> p c t", c=NDC),
        )
        nc.sync.dma_start(outd[_ts(i), 0:D], xn[:])

    # --- Q' projection for one 512-wide t-span ---
    def qproj(s):
        for m in range(NDC):
            pq = pQVp.tile([128, 512], f32, name=f"pq{s}_{m}", tag="pq")
            for dc in range(NDC):
                nc.tensor.matmul(
                    pq[:],
                    Gw[:, dc, _ts(m)],
                    xTp[:, dc, PAD + 512 * s : PAD + 512 * (s + 1)],
                    start=(dc == 0),
                    stop=(dc == NDC - 1),
                )
            nc.scalar.copy(QT[:, m, _ts(s, 512)], pq[:])

    # --- V projection for one 128-row chunk ---
    def vproj(i):
        pv = pQVp.tile([128, 512], f32, name=f"pv{i}", tag="pq")
        for dc in range(NDC):
            nc.tensor.matmul(
                pv[:],
                xTp[:, dc, PAD + 128 * i : PAD + 128 * (i + 1)],
                Vws[:, dc, :],
                start=(dc == 0),
                stop=(dc == NDC - 1),
            )
        nc.vector.tensor_copy(Vn[:, i, :], pv[:])
        if i > 0:
            nc.sync.dma_start(Vtail[:, i, :], Vn[113:128, i - 1, :])

    # --- banded attention for one 128-query chunk ---
    katt = int(os.environ.get("KATT", "7"))

    def attention(i):
        pl = pLp.tile([128, SPAN], f32, name=f"pl{i}", tag="pl")
        for dc in range(NDC):
            nc.tensor.matmul(
                pl[:],
                QT[:, dc, _ts(i)],
                xTp[:, dc, 128 * i : 128 * i + SPAN],
                start=(dc == 0),
                stop=(dc == NDC - 1),
            )
        if katt < 2:
            junk = smp.tile([128, SPAN], f32, name=f"jk{i}", tag="lm")
            nc.vector.tensor_copy(junk[:], pl[:])
            return
        Lm = smp.tile([128, SPAN], f32, name=f"lm{i}", tag="lm")
        nc.vector.tensor_add(Lm[:], pl[:], band[:])
        negm = smp.tile([128, 1], f32, name=f"nm{i}", tag="nm")
        nc.vector.reduce_max(
            negm[:], Lm[:], axis=mybir.AxisListType.X, negate=True
        )
        if katt < 3:
            return
        P = smp.tile([128, SPAN], f32, name=f"pp{i}", tag="pp")
        rowsum = smp.tile([128, 1], f32, name=f"rs{i}", tag="rs")
        nc.scalar.activation(P[:], Lm[:], AF.Exp, bias=negm[:], accum_out=rowsum[:])
        recip = smp.tile([128, 1], f32, name=f"rc{i}", tag="rc")
        nc.vector.reciprocal(recip[:], rowsum[:])
        if katt < 4:
            return
        pst0 = pTp.tile([PAD, 128], f32, name=f"ps0{i}", tag="pt")
        nc.tensor.transpose(pst0[:], P[:, 0:PAD], identity[:])
        pst1 = pTp.tile([128, 128], f32, name=f"ps1{i}", tag="pt")
        nc.tensor.transpose(pst1[:], P[:, PAD : PAD + 128], identity[:])
        if katt < 5:
            return
        st0 = smp.tile([PAD, 128], f32r, name=f"st0{i}", tag="st0")
        st1 = smp.tile([128, 128], f32r, name=f"st1{i}", tag="st1")
        nc.vector.tensor_copy(st0[:], pst0[:])
        nc.vector.tensor_copy(st1[:], pst1[:])
        if katt < 6:
            return
        pa = pAp.tile([128, 512], f32, name=f"pa{i}", tag="pa")
        nc.tensor.matmul(pa[:], st0[:], Vtail[:, i, :], start=True, stop=False)
        nc.tensor.matmul(pa[:], st1[:], Vn[:, i, :], start=False, stop=True)
        if katt < 7:
            return
        ans = smp.tile([128, 512], f32, name=f"ans{i}", tag="ans")
        nc.scalar.mul(ans[:], pa[:], recip[:])
        nc.sync.dma_start(outd[_ts(i), D : 2 * D], ans[:])

    phase = int(os.environ.get("KBISECT", "4"))
    for i in range(4):
        load_transpose(i)
    if phase >= 2:
        qproj(0)
    for i in range(4, NCH):
        load_transpose(i)
    if phase >= 2:
        qproj(1)
    if phase >= 3:
        for i in range(NCH):
            vproj(i)
    if phase >= 4:
        for i in range(NCH):
            attention(i)

    stack.close()


def _build():
    if "nc" in _cache:
        return _cache["nc"]
    nc = bacc.Bacc("TRN2", target_bir_lowering=False, debug=False, num_devices=B)
    xd = nc.dram_tensor("x", [T, D], f32, kind="ExternalInput")
    Gd = nc.dram_tensor("G", [D, D], f32, kind="ExternalInput")
    Vd = nc.dram_tensor("Vw", [D, D], f32, kind="ExternalInput")
    Bd = nc.dram_tensor("bandneg", [128, SPAN], f32, kind="ExternalInput")
    Zd = nc.dram_tensor("zeros", [128, D], f32, kind="ExternalInput")
    outd = nc.dram_tensor("out", [T, 2 * D], f32, kind="ExternalOutput")
    with tile.TileContext(nc) as tc:
        _emit(tc, nc, xd, Gd, Vd, Bd, Zd, outd)
    nc.compile()
    _cache["nc"] = nc
    return nc


def _band_mask():
    i = np.arange(128)[:, None]
    j = np.arange(SPAN)[None, :]
    return np.where((j >= i) & (j <= i + PAD), 0.0, MASKVAL).astype(np.float32)


def make_in_maps(inputs, M, C, V):
    x = np.ascontiguousarray(np.asarray(inputs, dtype=np.float32))
    M = np.asarray(M, dtype=np.float32)
    C = np.asarray(C, dtype=np.float32)
    V = np.ascontiguousarray(np.asarray(V, dtype=np.float32))
    assert x.shape == (B, T, D), x.shape
    G = np.ascontiguousarray(
        (M.astype(np.float64) @ C.astype(np.float64).T).astype(np.float32)
    )
    band = _band_mask()
    zeros = np.zeros((128, D), dtype=np.float32)
    return [
        {"x": x[b], "G": G, "Vw": V, "bandneg": band, "zeros": zeros}
        for b in range(B)
    ]


def kernel(inputs, M, C, V):
    nc = _build()
    in_maps = make_in_maps(inputs, M, C, V)
    res = run_bass_kernel_spmd(nc, in_maps, core_ids=list(range(B)))
    return np.stack([res.results[b]["out"] for b in range(B)], axis=0)


# revision 23
# speedup vs baseline: 1.1741x; 1.1741x over previous
"""Trainium2 Bass kernel for nn_AttentionCell (sparse local attention, W=16).

Contract: kernel(**inputs) takes the FULL inputs
    inputs: [8, 1024, 512] f32, M/C/V: [512, 512] f32
and returns the FULL output [8, 1024, 1024] f32
    out = concat([inputs, local_attention(inputs)], axis=-1)

Sharding: data-parallel over batch — one batch element per NeuronCore (8 cores).
M/C are fused on the host into G = M @ C.T so that
    logits = (x @ M) @ (x @ C).T = (x @ G) @ x.T
which removes the K projection entirely on device (keys are x itself).

Per-core device algorithm (x: [1024, 512]):
  1. xT = x.T via PE transposes, stored zero-padded by LEFT-1=15 columns on the
     left so every 128-query chunk's 143-wide key span is a contiguous slice.
  2. Q'T = G.T @ xT and Vn = x @ Vw as float32r matmuls (1 cyc/row on PE).
  3. Per 128-query chunk: banded logits L[i, j] (j over the 143-key span) as
     4 accumulating matmuls; softmax over the in-band 16 entries via an
     additive -1e9 band mask (out-of-sequence keys are zero columns of xT so
     their logits are exactly 0, matching the reference's zero-padding).
  4. S @ V via two matmuls (15-row tail from the previous V chunk + the
     aligned 128-row chunk) after transposing the scores on PE; the softmax
     normalization is folded into the PSUM->SBUF copy as a per-row scale.
"""

import os
import sys

import numpy as np

for _p in ("/opt/trn_rl_repo", "/opt/pypackages"):
    if os.path.isdir(_p) and _p not in sys.path:
        sys.path.append(_p)

import concourse.bacc as bacc
import concourse.tile as tile
from concourse import mybir
from concourse.bass_utils import run_bass_kernel_spmd
from concourse.masks import make_identity

f32 = mybir.dt.float32
f32r = mybir.dt.float32r

B = 8
T = 1024
D = 512
LEFT = 16
PAD = LEFT - 1  # 15
# fp32r matmuls require even innermost free counts (2 fp32 per cycle), so the
# per-chunk key span is padded 143 -> 144; the extra column is band-masked.
SPAN = 128 + PAD + 1  # 144
XTW = PAD + T + 1  # padded xT width: 15 zero cols left, 1 zero col right
NCH = T // 128  # query chunks per core
NDC = D // 128  # feature chunks
MASKVAL = -1.0e9

_cache: dict = {}


def _ts(i, n=128):
    return slice(i * n, (i + 1) * n)


def _emit(tc, nc, xd, Gd, Vd, Bd, Zd, outd):
    AF = mybir.ActivationFunctionType
    OP = mybir.AluOpType
    from contextlib import ExitStack

    stack = ExitStack()
    constp = stack.enter_context(tc.tile_pool(name="const", bufs=1))
    xinp = stack.enter_context(tc.tile_pool(name="xin", bufs=3))
    bigp = stack.enter_context(tc.tile_pool(name="big", bufs=1))
    smp = stack.enter_context(tc.tile_pool(name="sm", bufs=2))
    pTp = stack.enter_context(tc.tile_pool(name="pT", bufs=2, space="PSUM"))
    pQVp = stack.enter_context(tc.tile_pool(name="pQV", bufs=2, space="PSUM"))
    pLp = stack.enter_context(tc.tile_pool(name="pL", bufs=3, space="PSUM"))

    # --- constants / weights ---
    # Const DMAs go on the Scalar HWDGE queue so the Sync queue serves the
    # x-chunk loads immediately; V weights first (first consumer).
    identity = constp.tile([128, 128], f32)
    make_identity(nc, identity[:])
    Vws = constp.tile([128, NDC, D], f32r)
    nc.scalar.dma_start(Vws[:], Vd[:].rearrange("(c p) n -> p c n", p=128).bitcast(f32r))
    Gw = constp.tile([128, NDC, D], f32r)
    nc.scalar.dma_start(Gw[:], Gd[:].rearrange("(c p) n -> p c n", p=128).bitcast(f32r))
    band = constp.tile([128, SPAN], f32)
    nc.scalar.dma_start(band[:], Bd[:])

    # --- persistent activations ---
    # x.T, zero-padded: cols 0..14 (left halo) and col XTW-1 (right span pad).
    # memset cannot write float32r on trn2, so zero-fill comes from a small
    # DMA'd zeros input instead.
    xTp = bigp.tile([128, NDC, XTW], f32r)
    nc.scalar.dma_start(
        xTp[:, :, 0:PAD],
        Zd[:, 0 : NDC * PAD].rearrange("p (c t) -> p c t", c=NDC).bitcast(f32r),
    )
    nc.scalar.dma_start(
        xTp[:, :, XTW - 1 : XTW],
        Zd[:, 0:NDC].rearrange("p (c t) -> p c t", c=NDC).bitcast(f32r),
    )
    QT = bigp.tile([128, NDC, T], f32r)  # (x @ G).T
    Vn = bigp.tile([128, NCH, D], f32r)  # x @ Vw, natural layout
    Vtail = bigp.tile([PAD, NCH, D], f32r)  # V rows t0-15..t0-1 per chunk
    nc.scalar.dma_start(Vtail[:, 0, :], Zd[0:PAD, :].bitcast(f32r))

    # Passthrough half of the output: background DRAM->DRAM copy on the
    # (otherwise idle) GpSimd SWDGE queues; no SBUF dependency at all.
    nc.gpsimd.dma_start(outd[:, 0:D], xd[:, :])

    # --- load + transpose x ---
    def load_transpose(i):
        xn = xinp.tile([128, D], f32, name=f"xn{i}", tag="xn")
        nc.sync.dma_start(xn[:], xd[_ts(i), :])
        pst = pTp.tile([128, D], f32, name=f"pt{i}", tag="pt")
        for dc in range(NDC):
            nc.tensor.transpose(pst[:, _ts(dc)], xn[:, _ts(dc)], identity[:])
        nc.vector.tensor_copy(
            xTp[:, :, PAD + 128 * i : PAD + 128 * (i + 1)],
            pst[:].rearrange("p (c t) -> p c t", c=NDC),
        )

    # --- Q' projection for one 512-wide t-span ---
    def qproj(s):
        for m in range(NDC):
            pq = pQVp.tile([128, 512], f32, name=f"pq{s}_{m}", tag="pq")
            for dc in range(NDC):
                nc.tensor.matmul(
                    pq[:],
                    Gw[:, dc, _ts(m)],
                    xTp[:, dc, PAD + 512 * s : PAD + 512 * (s + 1)],
                    start=(dc == 0),
                    stop=(dc == NDC - 1),
                )
            nc.scalar.copy(QT[:, m, _ts(s, 512)], pq[:])

    # --- V projection for one 128-row chunk ---
    def vproj(i):
        pv = pQVp.tile([128, 512], f32, name=f"pv{i}", tag="pq")
        for dc in range(NDC):
            nc.tensor.matmul(
                pv[:],
                xTp[:, dc, PAD + 128 * i : PAD + 128 * (i + 1)],
                Vws[:, dc, :],
                start=(dc == 0),
                stop=(dc == NDC - 1),
            )
        nc.vector.tensor_copy(Vn[:, i, :], pv[:])
        if i > 0:
            nc.sync.dma_start(Vtail[:, i, :], Vn[113:128, i - 1, :])

    # --- banded attention for one 128-query chunk, software-pipelined:
    # logits(i+1) is emitted (and scheduled on PE) while chunk i's softmax
    # runs on DVE/ACT, so the PE stream never drains and HAM stays warm.
    pltiles = {}

    def logits(i):
        pl = pLp.tile([128, SPAN], f32, name=f"pl{i}", tag="pl")
        for dc in range(NDC):
            nc.tensor.matmul(
                pl[:],
                QT[:, dc, _ts(i)],
                xTp[:, dc, 128 * i : 128 * i + SPAN],
                start=(dc == 0),
                stop=(dc == NDC - 1),
            )
        pltiles[i] = pl

    def softsv(i):
        pl = pltiles.pop(i)
        Lm = smp.tile([128, SPAN], f32, name=f"lm{i}", tag="lm")
        nc.vector.tensor_add(Lm[:], pl[:], band[:])
        negm = smp.tile([128, 1], f32, name=f"nm{i}", tag="nm")
        nc.vector.reduce_max(
            negm[:], Lm[:], axis=mybir.AxisListType.X, negate=True
        )
        P = smp.tile([128, SPAN], f32, name=f"pp{i}", tag="pp")
        rowsum = smp.tile([128, 1], f32, name=f"rs{i}", tag="rs")
        nc.scalar.activation(P[:], Lm[:], AF.Exp, bias=negm[:], accum_out=rowsum[:])
        recip = smp.tile([128, 1], f32, name=f"rc{i}", tag="rc")
        nc.vector.reciprocal(recip[:], rowsum[:])
        pst0 = pTp.tile([PAD, 128], f32, name=f"ps0{i}", tag="pt")
        nc.tensor.transpose(pst0[:], P[:, 0:PAD], identity[:])
        pst1 = pTp.tile([128, 128], f32, name=f"ps1{i}", tag="pt")
        nc.tensor.transpose(pst1[:], P[:, PAD : PAD + 128], identity[:])
        st0 = smp.tile([PAD, 128], f32r, name=f"st0{i}", tag="st0")
        st1 = smp.tile([128, 128], f32r, name=f"st1{i}", tag="st1")
        nc.vector.tensor_copy(st0[:], pst0[:])
        nc.vector.tensor_copy(st1[:], pst1[:])
        pa = pQVp.tile([128, 512], f32, name=f"pa{i}", tag="pq")
        nc.tensor.matmul(pa[:], st0[:], Vtail[:, i, :], start=True, stop=False)
        nc.tensor.matmul(pa[:], st1[:], Vn[:, i, :], start=False, stop=True)
        ans = smp.tile([128, 512], f32, name=f"ans{i}", tag="ans")
        nc.scalar.mul(ans[:], pa[:], recip[:])
        nc.sync.dma_start(outd[_ts(i), D : 2 * D], ans[:])

    # Interleave V projections with the transposes so the PE stream mixes
    # real matmuls among transpose-mode ops (transposes alone do not keep
    # the HAM clock-gate open).
    for i in range(4):
        load_transpose(i)
        vproj(i)
    qproj(0)
    for i in range(4, NCH):
        load_transpose(i)
        vproj(i)
    qproj(1)
    logits(0)
    for i in range(NCH):
        if i + 1 < NCH:
            logits(i + 1)
        softsv(i)

    stack.close()


def _build():
    if "nc" in _cache:
        return _cache["nc"]
    nc = bacc.Bacc("TRN2", target_bir_lowering=False, debug=False, num_devices=B)
    xd = nc.dram_tensor("x", [T, D], f32, kind="ExternalInput")
    Gd = nc.dram_tensor("G", [D, D], f32, kind="ExternalInput")
    Vd = nc.dram_tensor("Vw", [D, D], f32, kind="ExternalInput")
    Bd = nc.dram_tensor("bandneg", [128, SPAN], f32, kind="ExternalInput")
    Zd = nc.dram_tensor("zeros", [128, D], f32, kind="ExternalInput")
    outd = nc.dram_tensor("out", [T, 2 * D], f32, kind="ExternalOutput")
    with tile.TileContext(nc) as tc:
        _emit(tc, nc, xd, Gd, Vd, Bd, Zd, outd)
    nc.compile()
    _cache["nc"] = nc
    return nc


def _band_mask():
    i = np.arange(128)[:, None]
    j = np.arange(SPAN)[None, :]
    return np.where((j >= i) & (j <= i + PAD), 0.0, MASKVAL).astype(np.float32)


def make_in_maps(inputs, M, C, V):
    x = np.ascontiguousarray(np.asarray(inputs, dtype=np.float32))
    M = np.asarray(M, dtype=np.float32)
    C = np.asarray(C, dtype=np.float32)
    V = np.ascontiguousarray(np.asarray(V, dtype=np.float32))
    assert x.shape == (B, T, D), x.shape
    G = np.ascontiguousarray(
        (M.astype(np.float64) @ C.astype(np.float64).T).astype(np.float32)
    )
    band = _band_mask()
    zeros = np.zeros((128, D), dtype=np.float32)
    return [
        {"x": x[b], "G": G, "Vw": V, "bandneg": band, "zeros": zeros}
        for b in range(B)
    ]


def kernel(inputs, M, C, V):
    nc = _build()
    in_maps = make_in_maps(inputs, M, C, V)
    res = run_bass_kernel_spmd(nc, in_maps, core_ids=list(range(B)))
    return np.stack([res.results[b]["out"] for b in range(B)], axis=0)
